# revision 37
# baseline (speedup 1.0000x reference)
"""AdditiveAttention (Bahdanau) distributed Bass kernel for 8 TRN2 NeuronCores.

Computation (per batch b):
    qc[b,:]   = query[b] @ Wq + bq + bv                       # [512]
    z[b,s,:]  = value[b,s] @ Wv + qc[b]                       # pre-tanh
    score     = tanh(z) @ Wo          (+bo dropped: cancels in softmax)
    align     = softmax(score)        (no max-sub: |score| <= ~23, exp fits f32)
    out[b,:]  = align @ value[b]

Sharding: data-parallel over batch, 4 batches per core, weights replicated.

v2 design (per core: B=4 batches, SEQ=4096, H=512), all bf16 compute:
  - value loaded HBM->SBUF with f32->bf16 cast DMA (SWDGE) in 2-block pair
    tiles v_pair[128p, 2blk, 4j, 512h], s = BLK*512 + p*4 + j (8KB DRAM
    runs, 2.1MB reads -> good DMA efficiency), prefetched one batch ahead.
  - one xbar DMA-transpose (HWDGE) per 512-seq block:
    [128, 2048] -> vT[128p, 16jj, 128s2], jj = j*4 + k, h = 128k + p.
  - main mm k-outer for stationary reuse: for (g=4-blk group, hoc, k):
    LDW(Wv[k,hoc]) then 4 accumulating MMs (one per blk) -> psum[128,512].
  - tanh on ACT with per-partition bias qcombT (query projection is free).
  - score MMs col-tiled: 4 blocks of a group write rows 0/32/64/96 of ONE
    psum bank via tile_position=(0,32*blk) -> they run concurrently on PE.
  - exp directly from score psum on ACT: esc2d[8blk, 512] bf16 (no [1,4096]
    single-partition exp, no DVE scrow copies).
  - escT via 8 selector matmuls (lhsT=esc97[g][:,128j:...], rhs=sel[g]) ->
    psum[128, 4, 8] -> one DVE copy (replaces 32 tiny MMs + 32 copies).
  - total = ones128^T @ escT (1 tiny MM) -> DVE reduce -> reciprocal.
  - context: 32 accumulating MMs escT[:,j,row]^T @ v_pair slice -> [1,512];
    scale by 1/total on DVE; DMA out.
  - batch tail (escT/tot/ctx/store) deferred into the NEXT batch's main mm
    stream so PE never drains; scores for (g,hoc) deferred one hoc slot so
    they never wait on tanh.
  - weights/query/biases loaded via HWDGE (scalar queue, parallel with
    SWDGE value stream) as f32 + DVE cast; setup MMs run before the first
    main MM on an otherwise idle PE; no nested pool-exit barriers (they
    poisoned the sync queue in the previous version, stalling the first
    transpose to t=53us).
"""

import numpy as np

N_CORES = 8
BATCH_TOTAL = 32
B = BATCH_TOTAL // N_CORES  # batches per core
SEQ = 4096
H = 512
HC = H // 128   # 4 hidden chunks
NBLK = SEQ // 512   # 8 seq blocks per batch
NPAIR = NBLK // 2   # pair-granular value loads

_cache = {}


def build_nc(b_per_core=B, seq=SEQ):
    import concourse.bass as bass
    import concourse.mybir as mybir
    import concourse.tile as tile
    from concourse import bacc
    from concourse.masks import make_identity

    f32 = mybir.dt.float32
    bf16 = mybir.dt.bfloat16
    AF = mybir.ActivationFunctionType
    AX = mybir.AxisListType
    ALU = mybir.AluOpType

    nblk = seq // 512
    npair = nblk // 2

    nc = bacc.Bacc("TRN2", target_bir_lowering=False, debug=False)

    val_d = nc.dram_tensor("value", [b_per_core, seq, H], f32, kind="ExternalInput").ap()
    q_d = nc.dram_tensor("query", [b_per_core, H], f32, kind="ExternalInput").ap()
    Wq_d = nc.dram_tensor("Wq", [H, H], f32, kind="ExternalInput").ap()
    bq_d = nc.dram_tensor("bq", [H], f32, kind="ExternalInput").ap()
    Wv_d = nc.dram_tensor("Wv", [H, H], f32, kind="ExternalInput").ap()
    bv_d = nc.dram_tensor("bv", [H], f32, kind="ExternalInput").ap()
    Wo_d = nc.dram_tensor("Wo", [H, 1], f32, kind="ExternalInput").ap()
    bo_d = nc.dram_tensor("bo", [1], f32, kind="ExternalInput").ap()  # unused (cancels)
    out_d = nc.dram_tensor("out", [b_per_core, H], f32, kind="ExternalOutput").ap()

    # s = (g*4 + blk)*512 + p*4 + j  -> [b, g, p, blk, j, h] group tiles
    val_v = val_d.rearrange(
        "b (g blk p j) h -> b g p blk j h", g=2, blk=4, p=128, j=4
    )
    # chunked weight rows (match xbar layout h = 128k + p)
    Wv_v = Wv_d.rearrange("(k p) o -> p k o", p=128)
    Wq_v = Wq_d.rearrange("(k p) o -> p k o", p=128)
    Wo_nat_v = Wo_d.rearrange("(r c) one -> r (c one)", c=128)  # [4, 128]
    bq_v = bq_d.rearrange("(r c) -> r c", c=128)                # [4, 128]
    bv_v = bv_d.rearrange("(r c) -> r c", c=128)

    with tile.TileContext(nc) as tc:
        with (
            tc.tile_pool(name="weights", bufs=1) as wpool,
            tc.tile_pool(name="wf32", bufs=1) as wfpool,
            tc.tile_pool(name="vnat", bufs=5) as vpool,
            tc.tile_pool(name="vt", bufs=6) as tpool,
            tc.tile_pool(name="ht", bufs=3) as hpool,
            tc.tile_pool(name="small", bufs=8) as smpool,
            tc.tile_pool(name="psum_h", bufs=4, space="PSUM") as psh,
            tc.tile_pool(name="psum_sc", bufs=2, space="PSUM") as pss,
            tc.tile_pool(name="psum_ctx", bufs=1, space="PSUM") as psc,
            tc.tile_pool(name="psum_e", bufs=1, space="PSUM") as pse_pool,
        ):
            # ---- issue the value loads for batch 0 first (critical path) ----
            vgroups = {}  # (b, g) -> tile [128, 4blk, 4j, 512h] bf16
            def load_batch(b, split=False):
                for g in range(2):
                    vt = vpool.tile([128, 4, 4, H], bf16, tag="vnat", name="vg")
                    if split:
                        nc.gpsimd.dma_start(out=vt[:, 0:2], in_=val_v[b, g, :, 0:2])
                        nc.gpsimd.dma_start(out=vt[:, 2:4], in_=val_v[b, g, :, 2:4])
                    else:
                        nc.gpsimd.dma_start(out=vt[:], in_=val_v[b, g])
                    vgroups[(b, g)] = vt

            # ---- persistent SBUF residents ----
            Wv_sb = wpool.tile([128, HC, H], bf16)
            Wq_sb = wpool.tile([128, HC, H], bf16)
            Wo_sb = wpool.tile([128, HC], bf16)
            qcombT = wpool.tile([128, HC, b_per_core], f32)
            ones128 = wpool.tile([128, 1], bf16)
            id4 = wpool.tile([4, 4], bf16)
            id4f = wpool.tile([4, 4], f32)
            qT = wpool.tile([128, HC, b_per_core], bf16)
            bqvT = wpool.tile([128, HC], f32)
            q_nat = wpool.tile([b_per_core, H], bf16)
            wo_nat = wpool.tile([4, 128], bf16)
            bq_s = wpool.tile([4, 128], f32)
            bv_s = wpool.tile([4, 128], f32)
            bqv = wpool.tile([4, 128], f32)
            # exp outputs per group: rows 0/32/64/96 (same partitions as the
            # col-tiled score psum rows -> no cross-partition ACT moves);
            # sel_g picks those rows out in the escT transpose matmuls.
            esc97 = [wpool.tile([97, H], bf16, name=f"esc97_{g}") for g in range(2)]
            sel = [wpool.tile([97, 8], bf16, name=f"sel_{g}") for g in range(2)]

            # value stream first on the SWDGE queue (critical path); the
            # setup loads ride the parallel HWDGE (scalar) ring as f32 and
            # get DVE-cast -- the early PE stall this causes is harmless
            # (PE has slack at the start), and it keeps the SWDGE queue
            # pure value traffic, which paces the whole pipeline.
            # batch-0 first half-chunk, then Wv as a SWDGE cast load (1MB,
            # ready ~10us, before the first transpose), then the rest of b0
            vt00 = vpool.tile([128, 4, 4, H], bf16, tag="vnat", name="vg00")
            nc.gpsimd.dma_start(out=vt00[:, 0:2], in_=val_v[0, 0, :, 0:2])
            vgroups[(0, 0)] = vt00
            nc.gpsimd.dma_start(out=Wv_sb[:], in_=Wv_v)
            nc.gpsimd.dma_start(out=vt00[:, 2:4], in_=val_v[0, 0, :, 2:4])
            vt01 = vpool.tile([128, 4, 4, H], bf16, tag="vnat", name="vg01")
            nc.gpsimd.dma_start(out=vt01[:, 0:2], in_=val_v[0, 1, :, 0:2])
            nc.gpsimd.dma_start(out=vt01[:, 2:4], in_=val_v[0, 1, :, 2:4])
            vgroups[(0, 1)] = vt01

            # scalar ring: Wq FIRST (the inline qcomb matmuls are the only
            # PE-FIFO blocker), then the tiny tensors
            wq_f32 = wfpool.tile([128, HC, H], f32, tag="wf32", name="wqf")
            nc.scalar.dma_start(out=wq_f32[:], in_=Wq_v)
            nc.vector.tensor_copy(Wq_sb[:], wq_f32[:])

            q_f32 = wfpool.tile([b_per_core, H], f32, tag="qf32")
            nc.scalar.dma_start(out=q_f32[:], in_=q_d)
            nc.vector.tensor_copy(q_nat[:], q_f32[:])

            wo_f32 = wfpool.tile([4, 128], f32, tag="wof32")
            nc.scalar.dma_start(out=wo_f32[:], in_=Wo_nat_v)
            nc.vector.tensor_copy(wo_nat[:], wo_f32[:])

            nc.scalar.dma_start(out=bq_s[:], in_=bq_v)
            nc.scalar.dma_start(out=bv_s[:], in_=bv_v)

            nc.gpsimd.memset(ones128[:], 1.0)
            make_identity(nc, id4[:])
            make_identity(nc, id4f[:])
            for g in range(2):
                nc.gpsimd.memset(esc97[g][:], 0.0)
                nc.gpsimd.memset(sel[g][:], 0.0)
                for a in range(4):
                    nc.gpsimd.memset(sel[g][32 * a:32 * a + 1, g * 4 + a:g * 4 + a + 1], 1.0)

            # prefetch batch 1 now that setup loads are queued
            if b_per_core > 1:
                load_batch(1)

            # Wo^T: PE-transpose [4,128] -> [128, 4]
            ps_wo = psh.tile([128, HC], f32, tag="ph", name="pswo")
            nc.tensor.matmul(ps_wo[:], wo_nat[:], id4[:], start=True, stop=True)
            nc.vector.tensor_copy(Wo_sb[:], ps_wo[:])

            # (bq+bv)^T via PE transpose (fp32, tiny)
            nc.vector.tensor_add(bqv[:], bq_s[:], bv_s[:])
            ps_b = psh.tile([128, HC], f32, tag="ph", name="psb")
            nc.tensor.matmul(ps_b[:], bqv[:], id4f[:], start=True, stop=True)
            nc.vector.tensor_copy(bqvT[:], ps_b[:])

            # q^T chunks: [128, B] per hic
            for hic in range(HC):
                ps_q = psh.tile([128, b_per_core], f32, tag="ph", name="psq")
                nc.tensor.matmul(
                    ps_q[:], q_nat[0:b_per_core, 128 * hic:128 * (hic + 1)],
                    id4[0:b_per_core, 0:b_per_core], start=True, stop=True,
                )
                nc.vector.tensor_copy(qT[:, hic, :], ps_q[:])

            # qcombT[ho, b] = (q[b] @ Wq)[ho] + bq[ho] + bv[ho]
            if True:
                for hoc in range(HC):
                    ps_qp = psh.tile([128, b_per_core], f32, tag="ph", name="psqp")
                    for hic in range(HC):
                        nc.tensor.matmul(
                            ps_qp[:], Wq_sb[:, hic, 128 * hoc:128 * (hoc + 1)],
                            qT[:, hic, :], start=(hic == 0), stop=(hic == HC - 1),
                        )
                    nc.scalar.activation(
                        qcombT[:, hoc, :], ps_qp[:], AF.Identity,
                        bias=bqvT[:, hoc:hoc + 1],
                    )

            # ---------------- main pipeline ----------------
            # deferred-emit state
            pending_scores = None  # (b, g, hoc, hT_g, ps_sc_g)
            pending_exp = None     # (b, g, ps_sc_g)
            tails = {}             # b -> escT_sb tile

            def emit_scores_and_exp():
                nonlocal pending_scores, pending_exp
                if pending_scores is not None:
                    p_, hoc_, hT_p_, ps_sc_ = pending_scores
                    for bi in range(2):
                        row = 32 * ((2 * p_ + bi) % 4)
                        nc.tensor.matmul(
                            ps_sc_[row:row + 1, :],
                            Wo_sb[:, hoc_:hoc_ + 1],
                            hT_p_[:, hoc_, bi, :],
                            start=(hoc_ == 0), stop=(hoc_ == HC - 1),
                            tile_position=(0, row),
                        )
                    if hoc_ == HC - 1 and p_ % 2 == 1:
                        pending_exp = (p_ // 2, ps_sc_)
                    pending_scores = None
                if pending_exp is not None:
                    g_, ps_sc_ = pending_exp
                    for bb in range(4):
                        nc.scalar.activation(
                            esc97[g_][32 * bb:32 * bb + 1, :],
                            ps_sc_[32 * bb:32 * bb + 1, :], AF.Exp,
                        )
                    pending_exp = None

            def emit_tail_a(b_):
                """escT transposes + psum->sbuf copy for batch b_."""
                pse = pse_pool.tile([128, HC, 8], f32, tag="pse")
                for j in range(HC):
                    for g_ in range(2):
                        nc.tensor.matmul(
                            pse[:, j, :], esc97[g_][:, 128 * j:128 * (j + 1)],
                            sel[g_][:], start=(g_ == 0), stop=(g_ == 1),
                        )
                escT_sb = smpool.tile([128, HC, 8], bf16, tag="escT")
                nc.vector.tensor_copy(escT_sb[:], pse[:])
                tails[b_] = escT_sb

            def emit_tail_b(b_):
                """total + reciprocal + context + store for batch b_."""
                escT_sb = tails.pop(b_)
                tot_ps = pse_pool.tile([1, HC * 8], f32, tag="pse", name="totps")
                nc.tensor.matmul(
                    tot_ps[:], ones128[:],
                    escT_sb[:].rearrange("p j r -> p (j r)"),
                    start=True, stop=True,
                )
                tot1 = smpool.tile([1, 1], f32, tag="tot1")
                nc.vector.tensor_reduce(tot1[:], tot_ps[:], axis=AX.X, op=ALU.add)
                rec = smpool.tile([1, 1], f32, tag="rec")
                nc.vector.reciprocal(rec[:], tot1[:])
                ps_ctx = psc.tile([1, H], f32, tag="ctx")
                n = 0
                for blk in range(nblk):
                    for j in range(HC):
                        nc.tensor.matmul(
                            ps_ctx[:],
                            escT_sb[:, j, blk:blk + 1],
                            vgroups[(b_, blk // 4)][:, blk % 4, j, :],
                            start=(n == 0), stop=(n == nblk * HC - 1),
                        )
                        n += 1
                outrow = smpool.tile([1, H], f32, tag="outrow")
                nc.vector.tensor_scalar_mul(outrow[:], ps_ctx[:], rec[:])
                nc.gpsimd.dma_start(out=out_d[b_:b_ + 1, :], in_=outrow[:])
                # release value tiles of b_
                for g_ in range(2):
                    del vgroups[(b_, g_)]

            for b in range(b_per_core):
                if b + 2 < b_per_core:
                    load_batch(b + 2)
                # one xbar transpose per 2-block pair (1MB ops, short vT
                # lifetime so the sync queue stays ahead of the PE)
                vTps = []
                for p in range(4):
                    vT = tpool.tile([128, 8 * HC, 128], bf16, tag="vt")
                    nc.sync.dma_start_transpose(
                        out=vT[:],
                        in_=vgroups[(b, p // 2)][:, (p % 2) * 2:(p % 2) * 2 + 2],
                    )
                    vTps.append(vT)

                ps_sc_g = None
                for p in range(4):
                    hT_p = hpool.tile([128, HC, 2, H], bf16, tag="ht")
                    if p % 2 == 0:
                        ps_sc_g = pss.tile([128, H], f32, tag="sc")
                    # chunk c = bi*16 + j*4 + k in the pair transpose
                    vTv = vTps[p][:].rearrange(
                        "p (bi j k) s -> p bi k j s", bi=2, k=HC
                    )
                    for hoc in range(HC):
                        # main MMs: k-outer, stationary Wv[k,hoc] reused
                        # across the 2 blocks of the pair
                        ps2 = [
                            psh.tile([128, H], f32, tag="ph", name=f"ps2_{i}")
                            for i in range(2)
                        ]
                        for k in range(HC):
                            for bi in range(2):
                                nc.tensor.matmul(
                                    ps2[bi][:],
                                    Wv_sb[:, k, 128 * hoc:128 * (hoc + 1)],
                                    vTv[:, bi, k],
                                    start=(k == 0), stop=(k == HC - 1),
                                )
                        # deferred score MMs (previous slot) + exp
                        emit_scores_and_exp()
                        # tanh with query-projection bias
                        for bi in range(2):
                            nc.scalar.activation(
                                hT_p[:, hoc, bi, :], ps2[bi][:], AF.Tanh,
                                bias=qcombT[:, hoc, b:b + 1],
                            )
                        pending_scores = (p, hoc, hT_p, ps_sc_g)
                        # interleave previous batch's tail into this stream
                        if p == 0 and hoc == 1 and b > 0:
                            emit_tail_a(b - 1)
                        if p == 0 and hoc == 2 and (b - 1) in tails:
                            emit_tail_b(b - 1)

            # drain: last group's scores + exp, then last batch tail
            emit_scores_and_exp()
            emit_scores_and_exp()
            last = b_per_core - 1
            emit_tail_a(last)
            emit_tail_b(last)

    nc.compile()
    return nc


def kernel(**inputs):
    from concourse.bass_utils import run_bass_kernel_spmd

    key = "full"
    if key not in _cache:
        _cache[key] = build_nc()
    nc = _cache[key]

    query = np.asarray(inputs["query"], dtype=np.float32)   # [1, 32, 512]
    value = np.asarray(inputs["value"], dtype=np.float32)   # [32, 4096, 512]
    Wq = np.asarray(inputs["Wq"], dtype=np.float32)
    bq = np.asarray(inputs["bq"], dtype=np.float32)
    Wv = np.asarray(inputs["Wv"], dtype=np.float32)
    bv = np.asarray(inputs["bv"], dtype=np.float32)
    Wo = np.asarray(inputs["Wo"], dtype=np.float32)
    bo = np.asarray(inputs["bo"], dtype=np.float32)

    in_maps = []
    for i in range(N_CORES):
        sl = slice(B * i, B * (i + 1))
        in_maps.append({
            "value": np.ascontiguousarray(value[sl]),
            "query": np.ascontiguousarray(query[0, sl, :]),
            "Wq": Wq, "bq": bq, "Wv": Wv, "bv": bv, "Wo": Wo, "bo": bo,
        })

    res = run_bass_kernel_spmd(nc, in_maps, core_ids=list(range(N_CORES)))
    out = np.concatenate([res.results[i]["out"] for i in range(N_CORES)], axis=0)
    return out[:, None, :].astype(np.float32)  # [32, 1, 512]


# revision 38
# speedup vs baseline: 1.0509x; 1.0509x over previous
"""AdditiveAttention (Bahdanau) distributed Bass kernel for 8 TRN2 NeuronCores.

Computation (per batch b):
    qc[b,:]   = query[b] @ Wq + bq + bv                       # [512]
    z[b,s,:]  = value[b,s] @ Wv + qc[b]                       # pre-tanh
    score     = tanh(z) @ Wo          (+bo dropped: cancels in softmax)
    align     = softmax(score)        (no max-sub: |score| <= ~23, exp fits f32)
    out[b,:]  = align @ value[b]

Sharding: data-parallel over batch, 4 batches per core, weights replicated.

v2 design (per core: B=4 batches, SEQ=4096, H=512), all bf16 compute:
  - value loaded HBM->SBUF with f32->bf16 cast DMA (SWDGE) in 2-block pair
    tiles v_pair[128p, 2blk, 4j, 512h], s = BLK*512 + p*4 + j (8KB DRAM
    runs, 2.1MB reads -> good DMA efficiency), prefetched one batch ahead.
  - one xbar DMA-transpose (HWDGE) per 512-seq block:
    [128, 2048] -> vT[128p, 16jj, 128s2], jj = j*4 + k, h = 128k + p.
  - main mm k-outer for stationary reuse: for (g=4-blk group, hoc, k):
    LDW(Wv[k,hoc]) then 4 accumulating MMs (one per blk) -> psum[128,512].
  - tanh on ACT with per-partition bias qcombT (query projection is free).
  - score MMs col-tiled: 4 blocks of a group write rows 0/32/64/96 of ONE
    psum bank via tile_position=(0,32*blk) -> they run concurrently on PE.
  - exp directly from score psum on ACT: esc2d[8blk, 512] bf16 (no [1,4096]
    single-partition exp, no DVE scrow copies).
  - escT via 8 selector matmuls (lhsT=esc97[g][:,128j:...], rhs=sel[g]) ->
    psum[128, 4, 8] -> one DVE copy (replaces 32 tiny MMs + 32 copies).
  - total = ones128^T @ escT (1 tiny MM) -> DVE reduce -> reciprocal.
  - context: 32 accumulating MMs escT[:,j,row]^T @ v_pair slice -> [1,512];
    scale by 1/total on DVE; DMA out.
  - batch tail (escT/tot/ctx/store) deferred into the NEXT batch's main mm
    stream so PE never drains; scores for (g,hoc) deferred one hoc slot so
    they never wait on tanh.
  - weights/query/biases loaded via HWDGE (scalar queue, parallel with
    SWDGE value stream) as f32 + DVE cast; setup MMs run before the first
    main MM on an otherwise idle PE; no nested pool-exit barriers (they
    poisoned the sync queue in the previous version, stalling the first
    transpose to t=53us).
"""

import numpy as np

N_CORES = 8
BATCH_TOTAL = 32
B = BATCH_TOTAL // N_CORES  # batches per core
SEQ = 4096
H = 512
HC = H // 128   # 4 hidden chunks
NBLK = SEQ // 512   # 8 seq blocks per batch
NPAIR = NBLK // 2   # pair-granular value loads

_cache = {}


def build_nc(b_per_core=B, seq=SEQ):
    import concourse.bass as bass
    import concourse.mybir as mybir
    import concourse.tile as tile
    from concourse import bacc
    from concourse.masks import make_identity

    f32 = mybir.dt.float32
    bf16 = mybir.dt.bfloat16
    AF = mybir.ActivationFunctionType
    AX = mybir.AxisListType
    ALU = mybir.AluOpType

    nblk = seq // 512
    npair = nblk // 2

    nc = bacc.Bacc("TRN2", target_bir_lowering=False, debug=False)

    val_d = nc.dram_tensor("value", [b_per_core, seq, H], f32, kind="ExternalInput").ap()
    q_d = nc.dram_tensor("query", [b_per_core, H], f32, kind="ExternalInput").ap()
    Wq_d = nc.dram_tensor("Wq", [H, H], f32, kind="ExternalInput").ap()
    bq_d = nc.dram_tensor("bq", [H], f32, kind="ExternalInput").ap()
    Wv_d = nc.dram_tensor("Wv", [H, H], f32, kind="ExternalInput").ap()
    bv_d = nc.dram_tensor("bv", [H], f32, kind="ExternalInput").ap()
    Wo_d = nc.dram_tensor("Wo", [H, 1], f32, kind="ExternalInput").ap()
    bo_d = nc.dram_tensor("bo", [1], f32, kind="ExternalInput").ap()  # unused (cancels)
    out_d = nc.dram_tensor("out", [b_per_core, H], f32, kind="ExternalOutput").ap()

    # s = (g*4 + blk)*512 + p*4 + j  -> [b, g, p, blk, j, h] group tiles
    val_v = val_d.rearrange(
        "b (g blk p j) h -> b g p blk j h", g=2, blk=4, p=128, j=4
    )
    # chunked weight rows (match xbar layout h = 128k + p)
    Wv_v = Wv_d.rearrange("(k p) o -> p k o", p=128)
    Wq_v = Wq_d.rearrange("(k p) o -> p k o", p=128)
    Wo_nat_v = Wo_d.rearrange("(r c) one -> r (c one)", c=128)  # [4, 128]
    bq_v = bq_d.rearrange("(r c) -> r c", c=128)                # [4, 128]
    bv_v = bv_d.rearrange("(r c) -> r c", c=128)

    with tile.TileContext(nc) as tc:
        with (
            tc.tile_pool(name="weights", bufs=1) as wpool,
            tc.tile_pool(name="wf32", bufs=1) as wfpool,
            tc.tile_pool(name="vnat", bufs=5) as vpool,
            tc.tile_pool(name="vt", bufs=6) as tpool,
            tc.tile_pool(name="ht", bufs=3) as hpool,
            tc.tile_pool(name="small", bufs=8) as smpool,
            tc.tile_pool(name="psum_h", bufs=4, space="PSUM") as psh,
            tc.tile_pool(name="psum_sc", bufs=2, space="PSUM") as pss,
            tc.tile_pool(name="psum_ctx", bufs=1, space="PSUM") as psc,
            tc.tile_pool(name="psum_e", bufs=1, space="PSUM") as pse_pool,
        ):
            # ---- issue the value loads for batch 0 first (critical path) ----
            vgroups = {}  # (b, g) -> tile [128, 4blk, 4j, 512h] bf16
            def load_batch(b, split=False):
                for g in range(2):
                    vt = vpool.tile([128, 4, 4, H], bf16, tag="vnat", name="vg")
                    if split:
                        nc.gpsimd.dma_start(out=vt[:, 0:2], in_=val_v[b, g, :, 0:2])
                        nc.gpsimd.dma_start(out=vt[:, 2:4], in_=val_v[b, g, :, 2:4])
                    else:
                        nc.gpsimd.dma_start(out=vt[:], in_=val_v[b, g])
                    vgroups[(b, g)] = vt

            # ---- persistent SBUF residents ----
            Wv_sb = wpool.tile([128, HC, H], bf16)
            Wq_sb = wpool.tile([128, HC, H], bf16)
            Wo_sb = wpool.tile([128, HC], bf16)
            qcombT = wpool.tile([128, HC, b_per_core], f32)
            ones128 = wpool.tile([128, 1], bf16)
            id4 = wpool.tile([4, 4], bf16)
            id4f = wpool.tile([4, 4], f32)
            qT = wpool.tile([128, HC, b_per_core], bf16)
            bqvT = wpool.tile([128, HC], f32)
            q_nat = wpool.tile([b_per_core, H], bf16)
            wo_nat = wpool.tile([4, 128], bf16)
            bq_s = wpool.tile([4, 128], f32)
            bv_s = wpool.tile([4, 128], f32)
            bqv = wpool.tile([4, 128], f32)
            # exp outputs per group: rows 0/32/64/96 (same partitions as the
            # col-tiled score psum rows -> no cross-partition ACT moves);
            # sel_g picks those rows out in the escT transpose matmuls.
            esc97 = [wpool.tile([97, H], bf16, name=f"esc97_{g}") for g in range(2)]
            sel = [wpool.tile([97, 8], bf16, name=f"sel_{g}") for g in range(2)]

            # value stream first on the SWDGE queue (critical path); the
            # setup loads ride the parallel HWDGE (scalar) ring as f32 and
            # get DVE-cast -- the early PE stall this causes is harmless
            # (PE has slack at the start), and it keeps the SWDGE queue
            # pure value traffic, which paces the whole pipeline.
            # batch-0 first half-chunk, then Wv as a SWDGE cast load (1MB,
            # ready ~10us, before the first transpose), then the rest of b0
            vt00 = vpool.tile([128, 4, 4, H], bf16, tag="vnat", name="vg00")
            nc.gpsimd.dma_start(out=vt00[:, 0:2], in_=val_v[0, 0, :, 0:2])
            vgroups[(0, 0)] = vt00
            nc.gpsimd.dma_start(out=Wv_sb[:], in_=Wv_v)
            nc.gpsimd.dma_start(out=vt00[:, 2:4], in_=val_v[0, 0, :, 2:4])
            vt01 = vpool.tile([128, 4, 4, H], bf16, tag="vnat", name="vg01")
            nc.gpsimd.dma_start(out=vt01[:, 0:2], in_=val_v[0, 1, :, 0:2])
            nc.gpsimd.dma_start(out=vt01[:, 2:4], in_=val_v[0, 1, :, 2:4])
            vgroups[(0, 1)] = vt01

            # scalar ring: Wq FIRST (the inline qcomb matmuls are the only
            # PE-FIFO blocker), then the tiny tensors
            wq_f32 = wfpool.tile([128, HC, H], f32, tag="wf32", name="wqf")
            nc.scalar.dma_start(out=wq_f32[:], in_=Wq_v)
            nc.vector.tensor_copy(Wq_sb[:], wq_f32[:])

            q_f32 = wfpool.tile([b_per_core, H], f32, tag="qf32")
            nc.scalar.dma_start(out=q_f32[:], in_=q_d)
            nc.vector.tensor_copy(q_nat[:], q_f32[:])

            wo_f32 = wfpool.tile([4, 128], f32, tag="wof32")
            nc.scalar.dma_start(out=wo_f32[:], in_=Wo_nat_v)
            nc.vector.tensor_copy(wo_nat[:], wo_f32[:])

            nc.scalar.dma_start(out=bq_s[:], in_=bq_v)
            nc.scalar.dma_start(out=bv_s[:], in_=bv_v)

            nc.gpsimd.memset(ones128[:], 1.0)
            make_identity(nc, id4[:])
            make_identity(nc, id4f[:])
            for g in range(2):
                nc.gpsimd.memset(esc97[g][:], 0.0)
                nc.gpsimd.memset(sel[g][:], 0.0)
                for a in range(4):
                    nc.gpsimd.memset(sel[g][32 * a:32 * a + 1, g * 4 + a:g * 4 + a + 1], 1.0)

            # prefetch batch 1 now that setup loads are queued
            if b_per_core > 1:
                load_batch(1)

            # Wo^T: PE-transpose [4,128] -> [128, 4]
            ps_wo = psh.tile([128, HC], f32, tag="ph", name="pswo")
            nc.tensor.matmul(ps_wo[:], wo_nat[:], id4[:], start=True, stop=True)
            nc.vector.tensor_copy(Wo_sb[:], ps_wo[:])

            # (bq+bv)^T via PE transpose (fp32, tiny)
            nc.vector.tensor_add(bqv[:], bq_s[:], bv_s[:])
            ps_b = psh.tile([128, HC], f32, tag="ph", name="psb")
            nc.tensor.matmul(ps_b[:], bqv[:], id4f[:], start=True, stop=True)
            nc.vector.tensor_copy(bqvT[:], ps_b[:])

            # q^T chunks: [128, B] per hic
            for hic in range(HC):
                ps_q = psh.tile([128, b_per_core], f32, tag="ph", name="psq")
                nc.tensor.matmul(
                    ps_q[:], q_nat[0:b_per_core, 128 * hic:128 * (hic + 1)],
                    id4[0:b_per_core, 0:b_per_core], start=True, stop=True,
                )
                nc.vector.tensor_copy(qT[:, hic, :], ps_q[:])

            # qcombT[ho, b] = (q[b] @ Wq)[ho] + bq[ho] + bv[ho]
            if True:
                for hoc in range(HC):
                    ps_qp = psh.tile([128, b_per_core], f32, tag="ph", name="psqp")
                    for hic in range(HC):
                        nc.tensor.matmul(
                            ps_qp[:], Wq_sb[:, hic, 128 * hoc:128 * (hoc + 1)],
                            qT[:, hic, :], start=(hic == 0), stop=(hic == HC - 1),
                        )
                    nc.scalar.activation(
                        qcombT[:, hoc, :], ps_qp[:], AF.Identity,
                        bias=bqvT[:, hoc:hoc + 1],
                    )

            # ---------------- main pipeline ----------------
            # deferred-emit state
            pending_scores = None  # (b, g, hoc, hT_g, ps_sc_g)
            pending_exp = None     # (b, g, ps_sc_g)
            tails = {}             # b -> escT_sb tile

            def emit_scores_and_exp():
                nonlocal pending_scores, pending_exp
                if pending_scores is not None:
                    p_, hoc_, hT_p_, ps_sc_ = pending_scores
                    for bi in range(2):
                        row = 32 * ((2 * p_ + bi) % 4)
                        nc.tensor.matmul(
                            ps_sc_[row:row + 1, :],
                            Wo_sb[:, hoc_:hoc_ + 1],
                            hT_p_[:, hoc_, bi, :],
                            start=(hoc_ == 0), stop=(hoc_ == HC - 1),
                            tile_position=(0, row),
                        )
                    if hoc_ == HC - 1 and p_ % 2 == 1:
                        pending_exp = (p_ // 2, ps_sc_)
                    pending_scores = None
                if pending_exp is not None:
                    g_, ps_sc_ = pending_exp
                    for bb in range(4):
                        nc.scalar.activation(
                            esc97[g_][32 * bb:32 * bb + 1, :],
                            ps_sc_[32 * bb:32 * bb + 1, :], AF.Exp,
                        )
                    pending_exp = None

            def emit_tail_a(b_):
                """escT transposes + psum->sbuf copy for batch b_."""
                pse = pse_pool.tile([128, HC, 8], f32, tag="pse")
                for j in range(HC):
                    for g_ in range(2):
                        nc.tensor.matmul(
                            pse[:, j, :], esc97[g_][:, 128 * j:128 * (j + 1)],
                            sel[g_][:], start=(g_ == 0), stop=(g_ == 1),
                        )
                escT_sb = smpool.tile([128, HC, 8], bf16, tag="escT")
                nc.vector.tensor_copy(escT_sb[:], pse[:])
                tails[b_] = escT_sb

            def emit_tail_b(b_):
                """total + reciprocal + context + store for batch b_."""
                escT_sb = tails.pop(b_)
                tot_ps = pse_pool.tile([1, HC * 8], f32, tag="pse", name="totps")
                nc.tensor.matmul(
                    tot_ps[:], ones128[:],
                    escT_sb[:].rearrange("p j r -> p (j r)"),
                    start=True, stop=True,
                )
                tot1 = smpool.tile([1, 1], f32, tag="tot1")
                nc.vector.tensor_reduce(tot1[:], tot_ps[:], axis=AX.X, op=ALU.add)
                rec = smpool.tile([1, 1], f32, tag="rec")
                nc.vector.reciprocal(rec[:], tot1[:])
                ps_ctx = psc.tile([1, H], f32, tag="ctx")
                n = 0
                for blk in range(nblk):
                    for j in range(HC):
                        nc.tensor.matmul(
                            ps_ctx[:],
                            escT_sb[:, j, blk:blk + 1],
                            vgroups[(b_, blk // 4)][:, blk % 4, j, :],
                            start=(n == 0), stop=(n == nblk * HC - 1),
                        )
                        n += 1
                outrow = smpool.tile([1, H], f32, tag="outrow")
                nc.vector.tensor_scalar_mul(outrow[:], ps_ctx[:], rec[:])
                nc.gpsimd.dma_start(out=out_d[b_:b_ + 1, :], in_=outrow[:])
                # release value tiles of b_
                for g_ in range(2):
                    del vgroups[(b_, g_)]

            for b in range(b_per_core):
                # one xbar transpose per 2-block pair (1MB ops, short vT
                # lifetime so the sync queue stays ahead of the PE);
                # the b+2 prefetch loads are deferred to mid-batch so the
                # SDMA engines give this batch's transposes full bandwidth
                vTps = []
                for p in range(4):
                    vT = tpool.tile([128, 8 * HC, 128], bf16, tag="vt")
                    nc.sync.dma_start_transpose(
                        out=vT[:],
                        in_=vgroups[(b, p // 2)][:, (p % 2) * 2:(p % 2) * 2 + 2],
                    )
                    vTps.append(vT)

                ps_sc_g = None
                for p in range(4):
                    hT_p = hpool.tile([128, HC, 2, H], bf16, tag="ht")
                    if p % 2 == 0:
                        ps_sc_g = pss.tile([128, H], f32, tag="sc")
                    # chunk c = bi*16 + j*4 + k in the pair transpose
                    vTv = vTps[p][:].rearrange(
                        "p (bi j k) s -> p bi k j s", bi=2, k=HC
                    )
                    for hoc in range(HC):
                        # main MMs: k-outer, stationary Wv[k,hoc] reused
                        # across the 2 blocks of the pair
                        ps2 = [
                            psh.tile([128, H], f32, tag="ph", name=f"ps2_{i}")
                            for i in range(2)
                        ]
                        for k in range(HC):
                            for bi in range(2):
                                nc.tensor.matmul(
                                    ps2[bi][:],
                                    Wv_sb[:, k, 128 * hoc:128 * (hoc + 1)],
                                    vTv[:, bi, k],
                                    start=(k == 0), stop=(k == HC - 1),
                                )
                        # deferred score MMs (previous slot) + exp
                        emit_scores_and_exp()
                        # tanh with query-projection bias
                        for bi in range(2):
                            nc.scalar.activation(
                                hT_p[:, hoc, bi, :], ps2[bi][:], AF.Tanh,
                                bias=qcombT[:, hoc, b:b + 1],
                            )
                        pending_scores = (p, hoc, hT_p, ps_sc_g)
                        # interleave previous batch's tail into this stream
                        if p == 0 and hoc == 1 and b > 0:
                            emit_tail_a(b - 1)
                        if p == 0 and hoc == 2 and (b - 1) in tails:
                            emit_tail_b(b - 1)
                        if p == 1 and hoc == 0 and b + 2 < b_per_core:
                            load_batch(b + 2)

            # drain: last group's scores + exp, then last batch tail
            emit_scores_and_exp()
            emit_scores_and_exp()
            last = b_per_core - 1
            emit_tail_a(last)
            emit_tail_b(last)

    nc.compile()
    return nc


def kernel(**inputs):
    from concourse.bass_utils import run_bass_kernel_spmd

    key = "full"
    if key not in _cache:
        _cache[key] = build_nc()
    nc = _cache[key]

    query = np.asarray(inputs["query"], dtype=np.float32)   # [1, 32, 512]
    value = np.asarray(inputs["value"], dtype=np.float32)   # [32, 4096, 512]
    Wq = np.asarray(inputs["Wq"], dtype=np.float32)
    bq = np.asarray(inputs["bq"], dtype=np.float32)
    Wv = np.asarray(inputs["Wv"], dtype=np.float32)
    bv = np.asarray(inputs["bv"], dtype=np.float32)
    Wo = np.asarray(inputs["Wo"], dtype=np.float32)
    bo = np.asarray(inputs["bo"], dtype=np.float32)

    in_maps = []
    for i in range(N_CORES):
        sl = slice(B * i, B * (i + 1))
        in_maps.append({
            "value": np.ascontiguousarray(value[sl]),
            "query": np.ascontiguousarray(query[0, sl, :]),
            "Wq": Wq, "bq": bq, "Wv": Wv, "bv": bv, "Wo": Wo, "bo": bo,
        })

    res = run_bass_kernel_spmd(nc, in_maps, core_ids=list(range(N_CORES)))
    out = np.concatenate([res.results[i]["out"] for i in range(N_CORES)], axis=0)
    return out[:, None, :].astype(np.float32)  # [32, 1, 512]


# revision 39
# speedup vs baseline: 1.0971x; 1.0440x over previous
"""AdditiveAttention (Bahdanau) distributed Bass kernel for 8 TRN2 NeuronCores.

Computation (per batch b):
    qc[b,:]   = query[b] @ Wq + bq + bv                       # [512]
    z[b,s,:]  = value[b,s] @ Wv + qc[b]                       # pre-tanh
    score     = tanh(z) @ Wo          (+bo dropped: cancels in softmax)
    align     = softmax(score)        (no max-sub: |score| <= ~23, exp fits f32)
    out[b,:]  = align @ value[b]

Sharding: data-parallel over batch, 4 batches per core, weights replicated.

v2 design (per core: B=4 batches, SEQ=4096, H=512), all bf16 compute:
  - value loaded HBM->SBUF with f32->bf16 cast DMA (SWDGE) in 2-block pair
    tiles v_pair[128p, 2blk, 4j, 512h], s = BLK*512 + p*4 + j (8KB DRAM
    runs, 2.1MB reads -> good DMA efficiency), prefetched one batch ahead.
  - one xbar DMA-transpose (HWDGE) per 512-seq block:
    [128, 2048] -> vT[128p, 16jj, 128s2], jj = j*4 + k, h = 128k + p.
  - main mm k-outer for stationary reuse: for (g=4-blk group, hoc, k):
    LDW(Wv[k,hoc]) then 4 accumulating MMs (one per blk) -> psum[128,512].
  - tanh on ACT with per-partition bias qcombT (query projection is free).
  - score MMs col-tiled: 4 blocks of a group write rows 0/32/64/96 of ONE
    psum bank via tile_position=(0,32*blk) -> they run concurrently on PE.
  - exp directly from score psum on ACT: esc2d[8blk, 512] bf16 (no [1,4096]
    single-partition exp, no DVE scrow copies).
  - escT via 8 selector matmuls (lhsT=esc97[g][:,128j:...], rhs=sel[g]) ->
    psum[128, 4, 8] -> one DVE copy (replaces 32 tiny MMs + 32 copies).
  - total = ones128^T @ escT (1 tiny MM) -> DVE reduce -> reciprocal.
  - context: 32 accumulating MMs escT[:,j,row]^T @ v_pair slice -> [1,512];
    scale by 1/total on DVE; DMA out.
  - batch tail (escT/tot/ctx/store) deferred into the NEXT batch's main mm
    stream so PE never drains; scores for (g,hoc) deferred one hoc slot so
    they never wait on tanh.
  - weights/query/biases loaded via HWDGE (scalar queue, parallel with
    SWDGE value stream) as f32 + DVE cast; setup MMs run before the first
    main MM on an otherwise idle PE; no nested pool-exit barriers (they
    poisoned the sync queue in the previous version, stalling the first
    transpose to t=53us).
"""

import numpy as np

N_CORES = 8
BATCH_TOTAL = 32
B = BATCH_TOTAL // N_CORES  # batches per core
SEQ = 4096
H = 512
HC = H // 128   # 4 hidden chunks
NBLK = SEQ // 512   # 8 seq blocks per batch
NPAIR = NBLK // 2   # pair-granular value loads

_cache = {}


def build_nc(b_per_core=B, seq=SEQ):
    import concourse.bass as bass
    import concourse.mybir as mybir
    import concourse.tile as tile
    from concourse import bacc
    from concourse.masks import make_identity

    f32 = mybir.dt.float32
    bf16 = mybir.dt.bfloat16
    AF = mybir.ActivationFunctionType
    AX = mybir.AxisListType
    ALU = mybir.AluOpType

    nblk = seq // 512
    npair = nblk // 2

    nc = bacc.Bacc("TRN2", target_bir_lowering=False, debug=False)

    val_d = nc.dram_tensor("value", [b_per_core, seq, H], f32, kind="ExternalInput").ap()
    q_d = nc.dram_tensor("query", [b_per_core, H], f32, kind="ExternalInput").ap()
    Wq_d = nc.dram_tensor("Wq", [H, H], f32, kind="ExternalInput").ap()
    bq_d = nc.dram_tensor("bq", [H], f32, kind="ExternalInput").ap()
    Wv_d = nc.dram_tensor("Wv", [H, H], f32, kind="ExternalInput").ap()
    bv_d = nc.dram_tensor("bv", [H], f32, kind="ExternalInput").ap()
    Wo_d = nc.dram_tensor("Wo", [H, 1], f32, kind="ExternalInput").ap()
    bo_d = nc.dram_tensor("bo", [1], f32, kind="ExternalInput").ap()  # unused (cancels)
    out_d = nc.dram_tensor("out", [b_per_core, H], f32, kind="ExternalOutput").ap()

    # s = (g*4 + blk)*512 + p*4 + j  -> [b, g, p, blk, j, h] group tiles
    val_v = val_d.rearrange(
        "b (g blk p j) h -> b g p blk j h", g=2, blk=4, p=128, j=4
    )
    # chunked weight rows (match xbar layout h = 128k + p)
    Wv_v = Wv_d.rearrange("(k p) o -> p k o", p=128)
    Wq_v = Wq_d.rearrange("(k p) o -> p k o", p=128)
    Wo_nat_v = Wo_d.rearrange("(r c) one -> r (c one)", c=128)  # [4, 128]
    bq_v = bq_d.rearrange("(r c) -> r c", c=128)                # [4, 128]
    bv_v = bv_d.rearrange("(r c) -> r c", c=128)

    with tile.TileContext(nc) as tc:
        with (
            tc.tile_pool(name="weights", bufs=1) as wpool,
            tc.tile_pool(name="wf32", bufs=1) as wfpool,
            tc.tile_pool(name="vnat", bufs=5) as vpool,
            tc.tile_pool(name="vt", bufs=6) as tpool,
            tc.tile_pool(name="ht", bufs=3) as hpool,
            tc.tile_pool(name="small", bufs=8) as smpool,
            tc.tile_pool(name="psum_h", bufs=4, space="PSUM") as psh,
            tc.tile_pool(name="psum_sc", bufs=2, space="PSUM") as pss,
            tc.tile_pool(name="psum_ctx", bufs=1, space="PSUM") as psc,
            tc.tile_pool(name="psum_e", bufs=1, space="PSUM") as pse_pool,
        ):
            # ---- issue the value loads for batch 0 first (critical path) ----
            vgroups = {}  # (b, g) -> tile [128, 4blk, 4j, 512h] bf16
            def load_batch(b, split=False):
                for g in range(2):
                    vt = vpool.tile([128, 4, 4, H], bf16, tag="vnat", name="vg")
                    if split:
                        nc.gpsimd.dma_start(out=vt[:, 0:2], in_=val_v[b, g, :, 0:2])
                        nc.gpsimd.dma_start(out=vt[:, 2:4], in_=val_v[b, g, :, 2:4])
                    else:
                        nc.gpsimd.dma_start(out=vt[:], in_=val_v[b, g])
                    vgroups[(b, g)] = vt

            def load_one(b, g):
                vt = vpool.tile([128, 4, 4, H], bf16, tag="vnat", name="vg1")
                nc.gpsimd.dma_start(out=vt[:], in_=val_v[b, g])
                vgroups[(b, g)] = vt

            # ---- persistent SBUF residents ----
            Wv_sb = wpool.tile([128, HC, H], bf16)
            Wq_sb = wpool.tile([128, HC, H], bf16)
            Wo_sb = wpool.tile([128, HC], bf16)
            qcombT = wpool.tile([128, HC, b_per_core], f32)
            ones128 = wpool.tile([128, 1], bf16)
            id4 = wpool.tile([4, 4], bf16)
            id4f = wpool.tile([4, 4], f32)
            qT = wpool.tile([128, HC, b_per_core], bf16)
            bqvT = wpool.tile([128, HC], f32)
            q_nat = wpool.tile([b_per_core, H], bf16)
            wo_nat = wpool.tile([4, 128], bf16)
            bq_s = wpool.tile([4, 128], f32)
            bv_s = wpool.tile([4, 128], f32)
            bqv = wpool.tile([4, 128], f32)
            # exp outputs per group: rows 0/32/64/96 (same partitions as the
            # col-tiled score psum rows -> no cross-partition ACT moves);
            # sel_g picks those rows out in the escT transpose matmuls.
            esc97 = [wpool.tile([97, H], bf16, name=f"esc97_{g}") for g in range(2)]
            sel = [wpool.tile([97, 8], bf16, name=f"sel_{g}") for g in range(2)]

            # value stream first on the SWDGE queue (critical path); the
            # setup loads ride the parallel HWDGE (scalar) ring as f32 and
            # get DVE-cast -- the early PE stall this causes is harmless
            # (PE has slack at the start), and it keeps the SWDGE queue
            # pure value traffic, which paces the whole pipeline.
            # batch-0 first half-chunk, then Wv as a SWDGE cast load (1MB,
            # ready ~10us, before the first transpose), then the rest of b0
            vt00 = vpool.tile([128, 4, 4, H], bf16, tag="vnat", name="vg00")
            nc.gpsimd.dma_start(out=vt00[:, 0:2], in_=val_v[0, 0, :, 0:2])
            vgroups[(0, 0)] = vt00
            nc.gpsimd.dma_start(out=Wv_sb[:], in_=Wv_v)
            nc.gpsimd.dma_start(out=vt00[:, 2:4], in_=val_v[0, 0, :, 2:4])
            vt01 = vpool.tile([128, 4, 4, H], bf16, tag="vnat", name="vg01")
            nc.gpsimd.dma_start(out=vt01[:, 0:2], in_=val_v[0, 1, :, 0:2])
            nc.gpsimd.dma_start(out=vt01[:, 2:4], in_=val_v[0, 1, :, 2:4])
            vgroups[(0, 1)] = vt01

            # scalar ring: Wq FIRST (the inline qcomb matmuls are the only
            # PE-FIFO blocker), then the tiny tensors
            wq_f32 = wfpool.tile([128, HC, H], f32, tag="wf32", name="wqf")
            nc.scalar.dma_start(out=wq_f32[:], in_=Wq_v)
            nc.vector.tensor_copy(Wq_sb[:], wq_f32[:])

            q_f32 = wfpool.tile([b_per_core, H], f32, tag="qf32")
            nc.scalar.dma_start(out=q_f32[:], in_=q_d)
            nc.vector.tensor_copy(q_nat[:], q_f32[:])

            wo_f32 = wfpool.tile([4, 128], f32, tag="wof32")
            nc.scalar.dma_start(out=wo_f32[:], in_=Wo_nat_v)
            nc.vector.tensor_copy(wo_nat[:], wo_f32[:])

            nc.scalar.dma_start(out=bq_s[:], in_=bq_v)
            nc.scalar.dma_start(out=bv_s[:], in_=bv_v)

            nc.gpsimd.memset(ones128[:], 1.0)
            make_identity(nc, id4[:])
            make_identity(nc, id4f[:])
            for g in range(2):
                nc.gpsimd.memset(esc97[g][:], 0.0)
                nc.gpsimd.memset(sel[g][:], 0.0)
                for a in range(4):
                    nc.gpsimd.memset(sel[g][32 * a:32 * a + 1, g * 4 + a:g * 4 + a + 1], 1.0)

            # prefetch batch 1 now that setup loads are queued
            if b_per_core > 1:
                load_batch(1)

            # Wo^T: PE-transpose [4,128] -> [128, 4]
            ps_wo = psh.tile([128, HC], f32, tag="ph", name="pswo")
            nc.tensor.matmul(ps_wo[:], wo_nat[:], id4[:], start=True, stop=True)
            nc.vector.tensor_copy(Wo_sb[:], ps_wo[:])

            # (bq+bv)^T via PE transpose (fp32, tiny)
            nc.vector.tensor_add(bqv[:], bq_s[:], bv_s[:])
            ps_b = psh.tile([128, HC], f32, tag="ph", name="psb")
            nc.tensor.matmul(ps_b[:], bqv[:], id4f[:], start=True, stop=True)
            nc.vector.tensor_copy(bqvT[:], ps_b[:])

            # q^T chunks: [128, B] per hic
            for hic in range(HC):
                ps_q = psh.tile([128, b_per_core], f32, tag="ph", name="psq")
                nc.tensor.matmul(
                    ps_q[:], q_nat[0:b_per_core, 128 * hic:128 * (hic + 1)],
                    id4[0:b_per_core, 0:b_per_core], start=True, stop=True,
                )
                nc.vector.tensor_copy(qT[:, hic, :], ps_q[:])

            # qcombT[ho, b] = (q[b] @ Wq)[ho] + bq[ho] + bv[ho]
            if True:
                for hoc in range(HC):
                    ps_qp = psh.tile([128, b_per_core], f32, tag="ph", name="psqp")
                    for hic in range(HC):
                        nc.tensor.matmul(
                            ps_qp[:], Wq_sb[:, hic, 128 * hoc:128 * (hoc + 1)],
                            qT[:, hic, :], start=(hic == 0), stop=(hic == HC - 1),
                        )
                    nc.scalar.activation(
                        qcombT[:, hoc, :], ps_qp[:], AF.Identity,
                        bias=bqvT[:, hoc:hoc + 1],
                    )

            # ---------------- main pipeline ----------------
            # deferred-emit state
            pending_scores = None  # (b, g, hoc, hT_g, ps_sc_g)
            pending_exp = None     # (b, g, ps_sc_g)
            tails = {}             # b -> escT_sb tile

            def emit_scores_and_exp():
                nonlocal pending_scores, pending_exp
                if pending_scores is not None:
                    p_, hoc_, hT_p_, ps_sc_ = pending_scores
                    for bi in range(2):
                        row = 32 * ((2 * p_ + bi) % 4)
                        nc.tensor.matmul(
                            ps_sc_[row:row + 1, :],
                            Wo_sb[:, hoc_:hoc_ + 1],
                            hT_p_[:, hoc_, bi, :],
                            start=(hoc_ == 0), stop=(hoc_ == HC - 1),
                            tile_position=(0, row),
                        )
                    if hoc_ == HC - 1 and p_ % 2 == 1:
                        pending_exp = (p_ // 2, ps_sc_)
                    pending_scores = None
                if pending_exp is not None:
                    g_, ps_sc_ = pending_exp
                    for bb in range(4):
                        nc.scalar.activation(
                            esc97[g_][32 * bb:32 * bb + 1, :],
                            ps_sc_[32 * bb:32 * bb + 1, :], AF.Exp,
                        )
                    pending_exp = None

            def emit_tail_a(b_):
                """escT transposes + psum->sbuf copy for batch b_."""
                pse = pse_pool.tile([128, HC, 8], f32, tag="pse")
                for j in range(HC):
                    for g_ in range(2):
                        nc.tensor.matmul(
                            pse[:, j, :], esc97[g_][:, 128 * j:128 * (j + 1)],
                            sel[g_][:], start=(g_ == 0), stop=(g_ == 1),
                        )
                escT_sb = smpool.tile([128, HC, 8], bf16, tag="escT")
                nc.vector.tensor_copy(escT_sb[:], pse[:])
                tails[b_] = escT_sb

            def emit_tail_b(b_):
                """total + reciprocal + context + store for batch b_."""
                escT_sb = tails.pop(b_)
                tot_ps = pse_pool.tile([1, HC * 8], f32, tag="pse", name="totps")
                nc.tensor.matmul(
                    tot_ps[:], ones128[:],
                    escT_sb[:].rearrange("p j r -> p (j r)"),
                    start=True, stop=True,
                )
                tot1 = smpool.tile([1, 1], f32, tag="tot1")
                nc.vector.tensor_reduce(tot1[:], tot_ps[:], axis=AX.X, op=ALU.add)
                rec = smpool.tile([1, 1], f32, tag="rec")
                nc.vector.reciprocal(rec[:], tot1[:])
                ps_ctx = psc.tile([1, H], f32, tag="ctx")
                n = 0
                for blk in range(nblk):
                    for j in range(HC):
                        nc.tensor.matmul(
                            ps_ctx[:],
                            escT_sb[:, j, blk:blk + 1],
                            vgroups[(b_, blk // 4)][:, blk % 4, j, :],
                            start=(n == 0), stop=(n == nblk * HC - 1),
                        )
                        n += 1
                outrow = smpool.tile([1, H], f32, tag="outrow")
                nc.vector.tensor_scalar_mul(outrow[:], ps_ctx[:], rec[:])
                nc.gpsimd.dma_start(out=out_d[b_:b_ + 1, :], in_=outrow[:])
                # release value tiles of b_
                for g_ in range(2):
                    del vgroups[(b_, g_)]

            for b in range(b_per_core):
                # one xbar transpose per 2-block pair (1MB ops, short vT
                # lifetime so the sync queue stays ahead of the PE);
                # the b+2 prefetch loads are deferred to mid-batch so the
                # SDMA engines give this batch's transposes full bandwidth
                vTps = []
                for p in range(4):
                    vT = tpool.tile([128, 8 * HC, 128], bf16, tag="vt")
                    nc.sync.dma_start_transpose(
                        out=vT[:],
                        in_=vgroups[(b, p // 2)][:, (p % 2) * 2:(p % 2) * 2 + 2],
                    )
                    vTps.append(vT)

                ps_sc_g = None
                for p in range(4):
                    hT_p = hpool.tile([128, HC, 2, H], bf16, tag="ht")
                    if p % 2 == 0:
                        ps_sc_g = pss.tile([128, H], f32, tag="sc")
                    # chunk c = bi*16 + j*4 + k in the pair transpose
                    vTv = vTps[p][:].rearrange(
                        "p (bi j k) s -> p bi k j s", bi=2, k=HC
                    )
                    for hoc in range(HC):
                        # main MMs: k-outer, stationary Wv[k,hoc] reused
                        # across the 2 blocks of the pair
                        ps2 = [
                            psh.tile([128, H], f32, tag="ph", name=f"ps2_{i}")
                            for i in range(2)
                        ]
                        for k in range(HC):
                            for bi in range(2):
                                nc.tensor.matmul(
                                    ps2[bi][:],
                                    Wv_sb[:, k, 128 * hoc:128 * (hoc + 1)],
                                    vTv[:, bi, k],
                                    start=(k == 0), stop=(k == HC - 1),
                                )
                        # deferred score MMs (previous slot) + exp
                        emit_scores_and_exp()
                        # tanh with query-projection bias
                        for bi in range(2):
                            nc.scalar.activation(
                                hT_p[:, hoc, bi, :], ps2[bi][:], AF.Tanh,
                                bias=qcombT[:, hoc, b:b + 1],
                            )
                        pending_scores = (p, hoc, hT_p, ps_sc_g)
                        # interleave previous batch's tail into this stream
                        if p == 0 and hoc == 1 and b > 0:
                            emit_tail_a(b - 1)
                        if p == 0 and hoc == 2 and (b - 1) in tails:
                            emit_tail_b(b - 1)
                        if p in (1, 2) and hoc == 0 and b + 2 < b_per_core:
                            load_one(b + 2, p - 1)

            # drain: last group's scores + exp, then last batch tail
            emit_scores_and_exp()
            emit_scores_and_exp()
            last = b_per_core - 1
            emit_tail_a(last)
            emit_tail_b(last)

    nc.compile()
    return nc


def kernel(**inputs):
    from concourse.bass_utils import run_bass_kernel_spmd

    key = "full"
    if key not in _cache:
        _cache[key] = build_nc()
    nc = _cache[key]

    query = np.asarray(inputs["query"], dtype=np.float32)   # [1, 32, 512]
    value = np.asarray(inputs["value"], dtype=np.float32)   # [32, 4096, 512]
    Wq = np.asarray(inputs["Wq"], dtype=np.float32)
    bq = np.asarray(inputs["bq"], dtype=np.float32)
    Wv = np.asarray(inputs["Wv"], dtype=np.float32)
    bv = np.asarray(inputs["bv"], dtype=np.float32)
    Wo = np.asarray(inputs["Wo"], dtype=np.float32)
    bo = np.asarray(inputs["bo"], dtype=np.float32)

    in_maps = []
    for i in range(N_CORES):
        sl = slice(B * i, B * (i + 1))
        in_maps.append({
            "value": np.ascontiguousarray(value[sl]),
            "query": np.ascontiguousarray(query[0, sl, :]),
            "Wq": Wq, "bq": bq, "Wv": Wv, "bv": bv, "Wo": Wo, "bo": bo,
        })

    res = run_bass_kernel_spmd(nc, in_maps, core_ids=list(range(N_CORES)))
    out = np.concatenate([res.results[i]["out"] for i in range(N_CORES)], axis=0)
    return out[:, None, :].astype(np.float32)  # [32, 1, 512]
